# revision 1
# baseline (speedup 1.0000x reference)
"""BotRGCN + MoE (top-1 of 2 experts) Trainium2 Bass kernel, 8-core SPMD.

Design:
  - Nodes sharded contiguously: core c owns nodes [c*6250, (c+1)*6250).
  - Activations processed in TRANSPOSED windows: (128 part = d-half, 2 d-chunks,
    <=512 nodes free); full transposed activations staged in per-core DRAM.
  - RGCN aggregation = aggregate-then-transform:
      s[seg] = sum_{e: seg_e=seg} x_full[src_e],  seg = rel*6250 + local_dst
      agg^T = sum_r W_rel[r]^T @ (s_r^T * cnt_inv)
    Segment sums via selector matmuls: per 128-edge chunk,
      psum[d_half, seg cols] += M_chunk(128 edges, d_half).T @ Sel(128 edges, gsize)
    with Sel = (iota == seg_local), built on DVE, 0/1 exact.
  - x tables for gathering are node-major in DRAM, AllGather'd across the
    8 cores per layer (the only collectives).
  - Gathers via gpsimd.dma_gather (int16 idx, lo/hi table split at 32768).
  - MoE via gated-h1: y^T = sum_e We2_e^T (g_e * lrelu(We1_e^T x + be1_e))
    + bias terms folded in as K=1 matmuls against the gate row.

Self-contained: hardcodes shapes; imports only installed packages.
"""

import numpy as np

N = 50000
E = 400000
D = 256
R = 2
NE = 2
OUT = 256
NCORES = 8
NLOC = N // NCORES  # 6250
WIN = 512
NWIN = (NLOC + WIN - 1) // WIN  # 13
LO_SPLIT = 32768
CHUNK = 128

# config
N_GRID = 128          # selector matmul column-group width
TABLE_F16 = False     # gather table + selector matmul in fp16
XFORM_F32R = False    # node-level matmuls in float32r

SELU_SCALE = 1.0507009873554805
SELU_ALPHA = 1.6732632423543772
NEG_SLOPE = 0.01


# ----------------------------------------------------------------------------
# host-side planning
# ----------------------------------------------------------------------------

def _wrap_idx(idx):
    """int16 index list (len multiple of 16) -> (128, len/16) wrapped+replicated."""
    n = len(idx)
    w = idx.reshape(n // 16, 16).T.astype(np.int16)
    return np.tile(w, (8, 1))


def build_plan(edge_index, edge_type):
    src = np.asarray(edge_index[0], dtype=np.int64)
    dst = np.asarray(edge_index[1], dtype=np.int64)
    rel = np.asarray(edge_type, dtype=np.int64)

    core = dst // NLOC
    seg = rel * NLOC + (dst % NLOC)
    NSEG = R * NLOC

    cnt = np.bincount((core * NSEG + seg).astype(np.int64),
                      minlength=NCORES * NSEG).reshape(NCORES, NSEG)

    # groups: (r, wbase, gbase, gsize, win_index)
    groups = []
    wi = 0
    for r in range(R):
        for w in range(NWIN):
            wb = w * WIN
            nw = min(WIN, NLOC - wb)
            g0 = 0
            while g0 < nw:
                gs = min(N_GRID, nw - g0)
                groups.append((r, wb, wb + g0, gs, wi))
                g0 += gs
            wi += 1
    n_windows_total = wi

    per_core = []
    for c in range(NCORES):
        m = core == c
        s_c, seg_c = src[m], seg[m]
        o = np.argsort(seg_c, kind="stable")
        per_core.append((s_c[o], seg_c[o]))

    n_lo = np.zeros(len(groups), np.int64)
    n_hi = np.zeros(len(groups), np.int64)
    core_group_edges = []
    for c in range(NCORES):
        s_c, seg_c = per_core[c]
        lst = []
        for gi, (r, wb, gb, gs, _) in enumerate(groups):
            lo_b = np.searchsorted(seg_c, r * NLOC + gb)
            hi_b = np.searchsorted(seg_c, r * NLOC + gb + gs)
            ss, gg = s_c[lo_b:hi_b], seg_c[lo_b:hi_b] - (r * NLOC + gb)
            is_lo = ss < LO_SPLIT
            lo = (ss[is_lo], gg[is_lo])
            hi = (ss[~is_lo] - LO_SPLIT, gg[~is_lo])
            lst.append((lo, hi))
            n_lo[gi] = max(n_lo[gi], -(-len(lo[0]) // CHUNK))
            n_hi[gi] = max(n_hi[gi], -(-len(hi[0]) // CHUNK))
        core_group_edges.append(lst)
    empty = (n_lo + n_hi) == 0
    n_lo[empty] = 1

    win_groups = [[] for _ in range(n_windows_total)]
    for gi, g in enumerate(groups):
        win_groups[g[4]].append(gi)

    slot_group = []
    win_slot_range = []
    for w in range(n_windows_total):
        start = len(slot_group)
        nlo_w = 0
        for gi in win_groups[w]:
            slot_group.extend([gi] * int(n_lo[gi]))
            nlo_w += int(n_lo[gi])
        nhi_w = 0
        for gi in win_groups[w]:
            slot_group.extend([gi] * int(n_hi[gi]))
            nhi_w += int(n_hi[gi])
        win_slot_range.append((start, nlo_w, nhi_w))
    n_slots = len(slot_group)

    idx_all = np.zeros((NCORES, n_slots, CHUNK), np.int16)
    seg_all = np.full((NCORES, CHUNK, n_slots), -1.0, np.float32)
    for c in range(NCORES):
        for w in range(n_windows_total):
            start, nlo_w, nhi_w = win_slot_range[w]
            cursor = start
            for half in (0, 1):
                for gi in win_groups[w]:
                    nsl = int((n_lo if half == 0 else n_hi)[gi])
                    if nsl == 0:
                        continue
                    ss, gg = core_group_edges[c][gi][half]
                    ne = len(ss)
                    pad = nsl * CHUNK - ne
                    ssp = np.concatenate([ss, np.zeros(pad, np.int64)])
                    ggp = np.concatenate([gg, np.full(pad, -1, np.int64)])
                    for k in range(nsl):
                        sl = cursor + k
                        idx_all[c, sl] = ssp[k * CHUNK:(k + 1) * CHUNK].astype(np.int16)
                        seg_all[c, :, sl] = ggp[k * CHUNK:(k + 1) * CHUNK].astype(np.float32)
                    cursor += nsl

    win_idx_cols = []
    col = 0
    for w in range(n_windows_total):
        start, nlo_w, nhi_w = win_slot_range[w]
        win_idx_cols.append((col, nlo_w * CHUNK, col + nlo_w * CHUNK // 16, nhi_w * CHUNK))
        col += (nlo_w + nhi_w) * CHUNK // 16
    tot_cols = col

    idx16 = []
    for c in range(NCORES):
        buf = np.zeros((128, tot_cols), np.int16)
        for w in range(n_windows_total):
            start, nlo_w, nhi_w = win_slot_range[w]
            c_lo, ni_lo, c_hi, ni_hi = win_idx_cols[w]
            if ni_lo:
                buf[:, c_lo:c_lo + ni_lo // 16] = _wrap_idx(
                    idx_all[c, start:start + nlo_w].reshape(-1))
            if ni_hi:
                buf[:, c_hi:c_hi + ni_hi // 16] = _wrap_idx(
                    idx_all[c, start + nlo_w:start + nlo_w + nhi_w].reshape(-1))
        idx16.append(buf)

    cntinv = np.ones((NCORES, n_windows_total, 1, WIN), np.float32)
    for c in range(NCORES):
        for r in range(R):
            for w in range(NWIN):
                wb = w * WIN
                nw = min(WIN, NLOC - wb)
                cc = cnt[c, r * NLOC + wb: r * NLOC + wb + nw]
                cntinv[c, r * NWIN + w, 0, :nw] = 1.0 / np.maximum(cc, 1)
    cntinv = np.broadcast_to(cntinv, (NCORES, n_windows_total, 128, WIN)).copy()

    plan = dict(
        groups=groups,
        slot_group=np.array(slot_group, np.int64),
        win_groups=win_groups,
        win_slot_range=win_slot_range,
        win_idx_cols=win_idx_cols,
        n_slots=n_slots,
        idx_cols=tot_cols,
        n_windows_total=n_windows_total,
        max_slots=max(ws[1] + ws[2] for ws in win_slot_range),
    )
    data = dict(idx16=idx16, seg_all=seg_all, cntinv=cntinv)
    return plan, data


# ----------------------------------------------------------------------------
# bass program
# ----------------------------------------------------------------------------

def build_nc(plan):
    import concourse.mybir as mybir
    import concourse.tile as tile
    from concourse import bacc
    from concourse.masks import make_identity

    dt = mybir.dt
    f32 = dt.float32
    tbl_dt = dt.float16 if TABLE_F16 else f32
    Alu = mybir.AluOpType
    ACT = mybir.ActivationFunctionType

    def xf(ap):
        return ap.bitcast(dt.float32r) if XFORM_F32R else ap

    NSEGW = plan["n_windows_total"]
    NSLOT = plan["n_slots"]
    IDXC = plan["idx_cols"]
    MAX_SLOTS = plan["max_slots"]
    groups = plan["groups"]
    slot_group = plan["slot_group"]

    nc = bacc.Bacc(None, num_devices=NCORES, num_swdge_queues=2)

    xcatT_in = nc.dram_tensor("xcatT", [128, 2, NLOC], f32, kind="ExternalInput")
    idx16_in = nc.dram_tensor("idx16", [128, IDXC], dt.int16, kind="ExternalInput")
    seg_in = nc.dram_tensor("segloc", [128, NSLOT], f32, kind="ExternalInput")
    cntinv_in = nc.dram_tensor("cntinv", [NSEGW, 128, WIN], f32, kind="ExternalInput")
    w_in_in = nc.dram_tensor("w_in", [128, 2, D], f32, kind="ExternalInput")
    w_root_in = nc.dram_tensor("w_root", [128, 2, D], f32, kind="ExternalInput")
    w_rel_in = nc.dram_tensor("w_rel", [R, 128, 2, D], f32, kind="ExternalInput")
    b_in_in = nc.dram_tensor("b_in", [128, 2], f32, kind="ExternalInput")
    b_rgcn_in = nc.dram_tensor("b_rgcn", [128, 2], f32, kind="ExternalInput")
    wg_in = nc.dram_tensor("wgate", [128, 2, 1], f32, kind="ExternalInput")
    we1_in = nc.dram_tensor("we1", [NE, 128, 2, D], f32, kind="ExternalInput")
    be1_in = nc.dram_tensor("be1", [NE, 128, 2], f32, kind="ExternalInput")
    we2_in = nc.dram_tensor("we2", [NE, 128, 2, OUT], f32, kind="ExternalInput")
    be2_in = nc.dram_tensor("be2row", [1, NE, 2, 128], f32, kind="ExternalInput")
    out_t = nc.dram_tensor("out", [OUT, NLOC], f32, kind="ExternalOutput")

    with tile.TileContext(nc) as tc:
        with (
            tc.tile_pool(name="const", bufs=1) as cpool,
            tc.tile_pool(name="work", bufs=2) as wpool,
            tc.tile_pool(name="slabp", bufs=3) as slabpool,
            tc.tile_pool(name="selp", bufs=4) as selpool,
            tc.tile_pool(name="stage", bufs=3) as stpool,
            tc.tile_pool(name="psum_sel", bufs=2, space="PSUM") as ps_sel,
            tc.tile_pool(name="psum_xf", bufs=2, space="PSUM") as ps_xf,
            tc.tile_pool(name="psum_misc", bufs=2, space="PSUM") as ps_misc,
            tc.tile_pool(name="dram", bufs=1, space="DRAM") as dpool,
            tc.tile_pool(name="dramsh", bufs=1, space="DRAM") as shpool,
        ):
            # constants / weights
            ident = cpool.tile([128, 128], f32)
            make_identity(nc, ident[:])
            iota_i = cpool.tile([128, N_GRID], dt.int32)
            nc.gpsimd.iota(iota_i[:], pattern=[[1, N_GRID]], base=0, channel_multiplier=0)
            iota_f = cpool.tile([128, N_GRID], f32)
            nc.vector.tensor_copy(iota_f[:], iota_i[:])
            ones_row = cpool.tile([1, 128], f32)
            nc.vector.memset(ones_row[:], 1.0)

            def load_const(t_in, shape, re=None, tag=None):
                t = cpool.tile(shape, f32, tag=tag)
                nc.sync.dma_start(t[:], t_in[:] if re is None else t_in[:].rearrange(re))
                return t

            w_in_sb = load_const(w_in_in, [128, 2, D], tag="w_in")
            w_root_sb = load_const(w_root_in, [128, 2, D], tag="w_root")
            w_rel_sb = load_const(w_rel_in, [128, R, 2, D], "r p k d -> p r k d",
                                  tag="w_rel")
            b_in_sb = load_const(b_in_in, [128, 2], tag="b_in")
            b_rg_sb = load_const(b_rgcn_in, [128, 2], tag="b_rg")
            wgd_sb = load_const(wg_in, [128, 2, 1], tag="wgd")
            we1_sb = load_const(we1_in, [128, NE, 2, D], "e p k d -> p e k d",
                                tag="we1")
            be1_sb = load_const(be1_in, [128, NE, 2], "e p k -> p e k", tag="be1")
            we2_sb = load_const(we2_in, [128, NE, 2, OUT], "e p k d -> p e k d",
                                tag="we2")
            be2_sb = cpool.tile([1, NE, 2, 128], f32)
            nc.sync.dma_start(be2_sb[:], be2_in[:])

            seg_sb = cpool.tile([128, NSLOT], f32)
            nc.sync.dma_start(seg_sb[:], seg_in[:])
            idx_sb = cpool.tile([128, IDXC], dt.int16)
            nc.sync.dma_start(idx_sb[:], idx16_in[:])

            # DRAM staging
            xT1 = dpool.tile([128, 2, NLOC], f32)
            xT2 = dpool.tile([128, 2, NLOC], f32)
            xloc1 = dpool.tile([NLOC, D], tbl_dt)
            xloc2 = dpool.tile([NLOC, D], tbl_dt)
            xfull1 = shpool.tile([N, D], tbl_dt, addr_space="Shared")
            xfull2 = shpool.tile([N, D], tbl_dt, addr_space="Shared")

            def win_sizes(w):
                wb = w * WIN
                return wb, min(WIN, NLOC - wb)

            def load_xwin(src_dram, wb, nw, tag):
                t = wpool.tile([128, 2, WIN], f32, tag=tag)
                nc.sync.dma_start(t[:, :, :nw], src_dram[:, :, wb:wb + nw])
                return t

            def export_window(xw, wb, nw, xloc):
                # transpose (128, 2, nw) -> node-major rows of xloc
                nb = 0
                while nb < nw:
                    bs = min(128, nw - nb)
                    stg = stpool.tile([128, D], tbl_dt, tag="stage")
                    for mc in range(2):
                        pst = ps_misc.tile([128, max(WIN, 128)], f32, space="PSUM",
                                           tag="misc")
                        nc.tensor.transpose(pst[:bs, :128], xw[:, mc, nb:nb + bs], ident[:])
                        nc.scalar.activation(stg[:bs, mc * 128:(mc + 1) * 128],
                                             pst[:bs, :128], ACT.Copy)
                    nc.sync.dma_start(xloc[wb + nb: wb + nb + bs, :], stg[:bs, :])
                    nb += bs

            # ---------------- layer 0: x1 = selu(x_cat @ W_in + b_in) -------
            for w in range(NWIN):
                wb, nw = win_sizes(w)
                xw = load_xwin(xcatT_in, wb, nw, "xw")
                xo = wpool.tile([128, 2, WIN], f32, tag="xo")
                for mc in range(2):
                    ps = ps_xf.tile([128, WIN], f32, space="PSUM", tag="xf")
                    for kc in range(2):
                        nc.tensor.matmul(
                            ps[:, :nw],
                            xf(w_in_sb[:, kc, mc * 128:(mc + 1) * 128]),
                            xf(xw[:, kc, :nw]),
                            start=(kc == 0), stop=(kc == 1),
                        )
                    pos = wpool.tile([128, WIN], f32, tag="selu_pos")
                    nc.vector.tensor_scalar(
                        out=pos[:, :nw], in0=ps[:, :nw],
                        scalar1=b_in_sb[:, mc:mc + 1], scalar2=0.0,
                        op0=Alu.add, op1=Alu.max)
                    neg = wpool.tile([128, WIN], f32, tag="selu_neg")
                    nc.vector.tensor_scalar(
                        out=neg[:, :nw], in0=ps[:, :nw],
                        scalar1=b_in_sb[:, mc:mc + 1], scalar2=0.0,
                        op0=Alu.add, op1=Alu.min)
                    e = wpool.tile([128, WIN], f32, tag="selu_e")
                    nc.scalar.activation(e[:, :nw], neg[:, :nw], ACT.Exp)
                    sa = SELU_SCALE * SELU_ALPHA
                    nc.vector.tensor_scalar(
                        out=e[:, :nw], in0=e[:, :nw], scalar1=sa, scalar2=sa,
                        op0=Alu.mult, op1=Alu.subtract)
                    nc.vector.tensor_scalar(
                        out=pos[:, :nw], in0=pos[:, :nw],
                        scalar1=SELU_SCALE, scalar2=None, op0=Alu.mult)
                    nc.vector.tensor_tensor(
                        out=xo[:, mc, :nw], in0=pos[:, :nw], in1=e[:, :nw], op=Alu.add)
                nc.sync.dma_start(xT1[:, :, wb:wb + nw], xo[:, :, :nw])
                export_window(xo, wb, nw, xloc1)

            nc.gpsimd.collective_compute(
                "AllGather", mybir.AluOpType.bypass,
                replica_groups=[list(range(NCORES))],
                ins=[xloc1[:].opt()], outs=[xfull1[:].opt()])

            # ---------------- rgcn layers ----------------
            def rgcn_layer(xfull, xT_cur, xT_next, xloc_next, li, moe_fn=None):
                for w in range(NWIN):
                    wb, nw = win_sizes(w)
                    s_tiles = {}
                    for r in range(R):
                        wi = r * NWIN + w
                        start_slot, nlo_w, nhi_w = plan["win_slot_range"][wi]
                        c_lo, ni_lo, c_hi, ni_hi = plan["win_idx_cols"][wi]
                        nslots_w = nlo_w + nhi_w
                        slab = slabpool.tile([128, MAX_SLOTS, D], tbl_dt, tag="slab")
                        if ni_lo:
                            nc.gpsimd.dma_gather(
                                out_ap=slab[:, :nlo_w, :],
                                in_ap=xfull[0:LO_SPLIT, :],
                                idxs_ap=idx_sb[:, c_lo:c_lo + ni_lo // 16],
                                num_idxs=ni_lo, num_idxs_reg=ni_lo, elem_size=D,
                                single_packet=False, queue_num=r)
                        if ni_hi:
                            nc.gpsimd.dma_gather(
                                out_ap=slab[:, nlo_w:nslots_w, :],
                                in_ap=xfull[LO_SPLIT:N, :],
                                idxs_ap=idx_sb[:, c_hi:c_hi + ni_hi // 16],
                                num_idxs=ni_hi, num_idxs_reg=ni_hi, elem_size=D,
                                single_packet=False, queue_num=r)
                        ps0 = ps_sel.tile([128, WIN], f32, space="PSUM", tag="sel0")
                        ps1 = ps_sel.tile([128, WIN], f32, space="PSUM", tag="sel1")
                        for k in range(nslots_w):
                            sl = start_slot + k
                            gi = int(slot_group[sl])
                            gb_in_win = groups[gi][2] - wb
                            gs = groups[gi][3]
                            sel = selpool.tile([128, N_GRID], tbl_dt, tag="sel")
                            nc.vector.tensor_scalar(
                                out=sel[:, :gs], in0=iota_f[:, :gs],
                                scalar1=seg_sb[:, sl:sl + 1], scalar2=None,
                                op0=Alu.is_equal)
                            cols = slice(gb_in_win, gb_in_win + gs)
                            nc.tensor.matmul(
                                ps0[:, cols], slab[:, k, 0:128], sel[:, :gs],
                                start=(k == 0), stop=(k == nslots_w - 1))
                            nc.tensor.matmul(
                                ps1[:, cols], slab[:, k, 128:256], sel[:, :gs],
                                start=(k == 0), stop=(k == nslots_w - 1))
                        ci = wpool.tile([128, WIN], f32, tag="cntinv")
                        nc.sync.dma_start(ci[:], cntinv_in[wi])
                        s0 = wpool.tile([128, WIN], f32, tag="s0_%d" % r)
                        s1 = wpool.tile([128, WIN], f32, tag="s1_%d" % r)
                        nc.vector.tensor_tensor(out=s0[:, :nw], in0=ps0[:, :nw],
                                                in1=ci[:, :nw], op=Alu.mult)
                        nc.vector.tensor_tensor(out=s1[:, :nw], in0=ps1[:, :nw],
                                                in1=ci[:, :nw], op=Alu.mult)
                        s_tiles[r] = (s0, s1)

                    xw = load_xwin(xT_cur, wb, nw, "xw")
                    xo = wpool.tile([128, 2, WIN], f32, tag="xo")
                    for mc in range(2):
                        ps = ps_xf.tile([128, WIN], f32, space="PSUM", tag="xf")
                        for kc in range(2):
                            nc.tensor.matmul(
                                ps[:, :nw],
                                xf(w_root_sb[:, kc, mc * 128:(mc + 1) * 128]),
                                xf(xw[:, kc, :nw]),
                                start=(kc == 0), stop=False)
                        for r in range(R):
                            for kc in range(2):
                                st = s_tiles[r][kc]
                                nc.tensor.matmul(
                                    ps[:, :nw],
                                    xf(w_rel_sb[:, r, kc, mc * 128:(mc + 1) * 128]),
                                    xf(st[:, :nw]),
                                    start=False, stop=(r == R - 1 and kc == 1))
                        nc.vector.tensor_scalar(
                            out=xo[:, mc, :nw], in0=ps[:, :nw],
                            scalar1=b_rg_sb[:, mc:mc + 1], scalar2=None, op0=Alu.add)
                    if xT_next is not None:
                        nc.sync.dma_start(xT_next[:, :, wb:wb + nw], xo[:, :, :nw])
                    if xloc_next is not None:
                        export_window(xo, wb, nw, xloc_next)
                    if moe_fn is not None:
                        moe_fn(xo, wb, nw)

            # ---------------- MoE (fused into layer 2 windows) ----------------
            def moe_window(xw, wb, nw):
                psl = ps_misc.tile([128, WIN], f32, space="PSUM", tag="misc")
                for kc in range(2):
                    nc.tensor.matmul(
                        psl[:1, :nw], wgd_sb[:, kc, :], xw[:, kc, :nw],
                        start=(kc == 0), stop=(kc == 1))
                g_row = wpool.tile([1, WIN], f32, tag="grow")
                nc.vector.tensor_scalar(out=g_row[:, :nw], in0=psl[:1, :nw],
                                        scalar1=0.0, scalar2=None, op0=Alu.is_ge)
                ginv_row = wpool.tile([1, WIN], f32, tag="ginvrow")
                nc.vector.tensor_scalar(out=ginv_row[:, :nw], in0=g_row[:, :nw],
                                        scalar1=-1.0, scalar2=1.0,
                                        op0=Alu.mult, op1=Alu.add)
                psb = ps_misc.tile([128, WIN], f32, space="PSUM", tag="misc")
                nc.tensor.matmul(psb[:, :nw], ones_row[:], g_row[:, :nw],
                                 start=True, stop=True)
                gb = wpool.tile([128, WIN], f32, tag="gb_sb")
                nc.scalar.activation(gb[:, :nw], psb[:, :nw], ACT.Copy)
                ginv = wpool.tile([128, WIN], f32, tag="ginv")
                nc.vector.tensor_scalar(out=ginv[:, :nw], in0=gb[:, :nw],
                                        scalar1=-1.0, scalar2=1.0,
                                        op0=Alu.mult, op1=Alu.add)

                h1g = {}
                for e in range(NE):
                    for mc in range(2):
                        psh = ps_xf.tile([128, WIN], f32, space="PSUM", tag="xf")
                        for kc in range(2):
                            nc.tensor.matmul(
                                psh[:, :nw],
                                xf(we1_sb[:, e, kc, mc * 128:(mc + 1) * 128]),
                                xf(xw[:, kc, :nw]),
                                start=(kc == 0), stop=(kc == 1))
                        h = wpool.tile([128, WIN], f32, tag="h1_%d_%d" % (e, mc))
                        nc.scalar.activation(
                            h[:, :nw], psh[:, :nw], ACT.Lrelu,
                            bias=be1_sb[:, e, mc:mc + 1], alpha=NEG_SLOPE)
                        gt = gb if e == 0 else ginv
                        nc.vector.tensor_tensor(out=h[:, :nw], in0=h[:, :nw],
                                                in1=gt[:, :nw], op=Alu.mult)
                        h1g[(e, mc)] = h
                for mc in range(2):
                    psy = ps_xf.tile([128, WIN], f32, space="PSUM", tag="xf")
                    first = True
                    for e in range(NE):
                        for kc in range(2):
                            nc.tensor.matmul(
                                psy[:, :nw],
                                xf(we2_sb[:, e, kc, mc * 128:(mc + 1) * 128]),
                                xf(h1g[(e, kc)][:, :nw]),
                                start=first, stop=False)
                            first = False
                    nc.tensor.matmul(psy[:, :nw], be2_sb[0:1, 0, mc, :],
                                     g_row[:, :nw], start=False, stop=False)
                    nc.tensor.matmul(psy[:, :nw], be2_sb[0:1, 1, mc, :],
                                     ginv_row[:, :nw], start=False, stop=True)
                    yt = wpool.tile([128, WIN], f32, tag="yt")
                    nc.scalar.activation(yt[:, :nw], psy[:, :nw], ACT.Copy)
                    nc.sync.dma_start(out_t[mc * 128:(mc + 1) * 128, wb:wb + nw],
                                      yt[:, :nw])

            rgcn_layer(xfull1, xT1, xT2, xloc2, 1)
            nc.gpsimd.collective_compute(
                "AllGather", mybir.AluOpType.bypass,
                replica_groups=[list(range(NCORES))],
                ins=[xloc2[:].opt()], outs=[xfull2[:].opt()])
            rgcn_layer(xfull2, xT2, None, None, 2, moe_fn=moe_window)

    nc.compile()
    return nc


# ----------------------------------------------------------------------------
# entry point
# ----------------------------------------------------------------------------

def kernel(des, tweet, num_prop, cat_prop, edge_index, edge_type,
           W_in, b_in, W_rel, W_root, b_rgcn, w_gate, We1, be1, We2, be2):
    from concourse.bass_utils import run_bass_kernel_spmd

    x_cat = np.concatenate(
        [np.asarray(des), np.asarray(tweet), np.asarray(num_prop),
         np.asarray(cat_prop)], axis=1).astype(np.float32)

    plan, data = build_plan(np.asarray(edge_index), np.asarray(edge_type))
    nc = build_nc(plan)

    def wmat(w):
        w = np.asarray(w, np.float32)
        return np.ascontiguousarray(w.reshape(2, 128, w.shape[1]).transpose(1, 0, 2))

    def bvec(b):
        return np.ascontiguousarray(np.asarray(b, np.float32).reshape(2, 128).T)

    w_rel_h = np.stack([wmat(np.asarray(W_rel)[r]) for r in range(R)])
    we1_h = np.stack([wmat(np.asarray(We1)[e]) for e in range(NE)])
    be1_h = np.stack([bvec(np.asarray(be1)[e]) for e in range(NE)])
    we2_h = np.stack([wmat(np.asarray(We2)[e]) for e in range(NE)])
    be2row = np.asarray(be2, np.float32).reshape(1, NE, 2, 128)

    in_maps = []
    for c in range(NCORES):
        xc = x_cat[c * NLOC:(c + 1) * NLOC]
        xcatT = np.ascontiguousarray(xc.T.reshape(2, 128, NLOC).transpose(1, 0, 2))
        in_maps.append({
            "xcatT": xcatT,
            "idx16": data["idx16"][c],
            "segloc": data["seg_all"][c],
            "cntinv": data["cntinv"][c],
            "w_in": wmat(W_in), "w_root": wmat(W_root), "w_rel": w_rel_h,
            "b_in": bvec(b_in), "b_rgcn": bvec(b_rgcn),
            "wgate": wmat(np.asarray(w_gate)[:, 0:1] - np.asarray(w_gate)[:, 1:2]),
            "we1": we1_h, "be1": be1_h,
            "we2": we2_h, "be2row": be2row,
        })

    res = run_bass_kernel_spmd(nc, in_maps, core_ids=list(range(NCORES)))
    global last_nc, last_in_maps
    last_nc, last_in_maps = nc, in_maps
    y = np.concatenate([res.results[c]["out"].T for c in range(NCORES)], axis=0)
    return y.astype(np.float32)


last_nc = None
last_in_maps = None

